# revision 42
# baseline (speedup 1.0000x reference)
"""Trainium2 Bass kernel for causal self-attention with clipped softmax.

Problem (hardcoded): B=2, S=2048, H=16, D=128, fp32 inputs.
    scores = (Q @ K^T) / sqrt(D), causal mask, p = softmax(scores)
    p = clip(1.06*p - 0.03, 0, 1)            # ZETA=1.03, GAMMA=-0.03
    out = p @ V

Sharding: 32 (batch, head) pairs -> 4 per core across 8 cores (tensor
parallel over heads + data parallel over batch). No cross-core comms.

Per-core device kernel (transposed-scores layout, all matmuls bf16):
  - inputs loaded natural [s, d] with fp32->bf16 cast during DMA (SWDGE)
  - Q, K transposed to [d, s] via single blocked xbar-transpose DMAs
    (3D out AP -> sixteen 128x128 block transposes per transfer)
  - scoresT[k, q] = K_tile-stationary @ QT-moving  (causal tiles only)
  - exp on ScalarE directly from PSUM; scale=1/sqrt(D) and a 1.06 factor
    (bias=ln 1.06) folded in, so E' = 1.06*E and Z' = sum_k E' = 1.06*Z
  - causal diagonal zeroing via GPSIMD affine_select
  - Z'[q] via ones-column matmuls accumulated in PSUM; scaled to
    zlo = (0.03/1.06)*Z' during the PSUM drain, then broadcast across
    all 128 partitions with GPSIMD partition_broadcast
  - clipped numerator in ONE custom fused DVE op:
        G = relu(min(E' - zlo, 33.333*zlo))  (= Z * clip(1.06p - 0.03, 0, 1))
  - outT[d, q] += V_tile-stationary @ G-moving (PSUM accumulation over k)
  - host unshard applies the final (0.03/zlo_row) scale + layout transpose
The per-pair stages are emitted software-pipelined (A=scores/exp,
B=rowsum/broadcast, C=clip/PV with per-group clip prefixes) so the
in-order engine queues never head-of-line block across pairs.
"""

import numpy as np

import concourse.bass as bass
import concourse.mybir as mybir
import concourse.tile as tile
from concourse import bacc, dve_ops
from concourse.bass_utils import run_bass_kernel_spmd
from concourse.dve_spec import Spec, Src0, Src1, C2, lower, minn, relu
from concourse.dve_spec import _has_src1 as has_src1
from concourse.dve_uop import DveOpSpec

B = 2
S = 2048
H = 16
D = 128
N_CORES = 8
NP = H * B // N_CORES  # (b,h) pairs per core = 4
NT = S // 128  # 128-col tiles along sequence = 16
INV_SQRT_D = 1.0 / np.sqrt(np.float64(D))
ZETA = 1.03
GAMMA = -0.03
ALPHA = ZETA - GAMMA  # 1.06
KHI = 1.0 / 0.03  # zhi = KHI * zlo

F32 = mybir.dt.float32
BF16 = mybir.dt.bfloat16


def _register_clip_op():
    """Custom fused DVE op: out = relu(min(in0 - in1, imm2*in1)).

    With in1 = zlo = (0.03/1.06)*Z' and imm2 = 1/0.03 this computes the
    clipped-softmax numerator G = min(max(E' - 0.03Z, 0), Z) in a single
    DVE pass (sub, mul-by-imm, min, relu: 4 ALU stages, 2 streams).
    """
    name = "CLIPQ_ANT"
    for op in dve_ops.OPS:
        if op.name == name:
            return op
    spec = Spec(
        body=relu(minn(Src0 - Src1, Src1 * C2)),
        reference=lambda in0, in1, s0, s1, imm2: np.maximum(
            np.minimum(in0 - in1, in1 * imm2), 0.0
        ).astype(np.float32),
    )
    row = dve_ops._CUSTOM_DVE_ROW_BASE + len(dve_ops.OPS)
    dve_ops._SUB_OPCODE_FOR_NAME[name] = row
    shas = {}
    for ver in ("v3", "v4"):
        try:
            lowered = DveOpSpec(
                name=name,
                opcode=row,
                uops=lower(spec, ver=ver),
                rd1_en=has_src1(spec),
            )
            shas[ver] = lowered.sha(ver)
        except Exception:  # noqa: BLE001 - v4 table gen may be unavailable
            pass
    op = dve_ops.DveOp(name, spec, subdim=False, uops_sha=shas)
    dve_ops.OPS.append(op)
    dve_ops.CUSTOM_DVE_SPECS[name] = spec
    return op


CLIPQ = _register_clip_op()


def build_core_program():
    """Build + compile the per-core SPMD program. Returns the Bacc module."""
    nc = bacc.Bacc(
        "TRN2", target_bir_lowering=False, debug=False, num_devices=N_CORES
    )

    q_d = nc.dram_tensor("q", [S, NP, D], F32, kind="ExternalInput").ap()
    k_d = nc.dram_tensor("k", [S, NP, D], F32, kind="ExternalInput").ap()
    v_d = nc.dram_tensor("v", [S, NP, D], F32, kind="ExternalInput").ap()
    out_t = nc.dram_tensor("out_t", [NP, D, S], F32, kind="ExternalOutput").ap()
    out_z = nc.dram_tensor("out_z", [NP, S], F32, kind="ExternalOutput").ap()

    with tile.TileContext(nc) as tc:
        Builder(tc, q_d, k_d, v_d, out_t, out_z).build()

    nc.compile()
    return nc


class Builder:
    def __init__(self, tc, q_d, k_d, v_d, out_t, out_z):
        self.tc = tc
        self.nc = tc.nc
        self.q_d, self.k_d, self.v_d = q_d, k_d, v_d
        self.out_t, self.out_z = out_t, out_z
        self.qt = [None] * NP
        self.kt = [None] * NP
        self.vn = [None] * NP
        self.et = [None] * NP  # per pair: list per kk
        self.zlo = [None] * NP

    def build(self):
        nc = self.nc
        with (
            self.tc.tile_pool(name="const", bufs=1) as constp,
            self.tc.tile_pool(name="nat", bufs=2) as natp,
            self.tc.tile_pool(name="vnp", bufs=3) as vnp,
            self.tc.tile_pool(name="tr", bufs=2) as trp,
            self.tc.tile_pool(name="et", bufs=3) as etp,
            self.tc.tile_pool(name="zb", bufs=2) as zbp,
            self.tc.tile_pool(name="osb", bufs=2) as osbp,
            self.tc.tile_pool(name="psS", bufs=2, space="PSUM") as psS,
            self.tc.tile_pool(name="psZ", bufs=2, space="PSUM") as psZ,
            self.tc.tile_pool(name="psO", bufs=2, space="PSUM") as psO,
        ):
            self.natp, self.vnp, self.trp, self.etp = natp, vnp, trp, etp
            self.zbp, self.osbp = zbp, osbp
            self.psS, self.psZ, self.psO = psS, psZ, psO

            self.ones_k = constp.tile([128, 1], BF16)
            nc.vector.memset(self.ones_k[:], 1.0)
            self.bias_ln = constp.tile([128, 1], F32)
            nc.vector.memset(self.bias_ln[:], float(np.log(ALPHA)))

            # software pipeline over pairs: A=scores/exp, B=Z/bcast, C=clip/PV
            self.stage_in(0)
            self.stage_in(1)
            self.stage_A(0)
            self.stage_in(2)
            self.stage_A(1)
            self.stage_B(0)
            self.stage_B(1)
            self.stage_in(3)
            self.stage_A(2)
            self.stage_C(0)
            self.stage_B(2)
            self.stage_A(3)
            self.stage_C(1)
            self.stage_B(3)
            self.stage_C(2)
            self.stage_C(3)

    def stage_in(self, j):
        nc = self.nc
        qn = self.natp.tile([128, S], BF16, tag="qn")
        kn = self.natp.tile([128, S], BF16, tag="kn")
        vn = self.vnp.tile([128, S], BF16, tag="vn")
        qt = self.trp.tile([128, S], BF16, tag="qt")
        kt = self.trp.tile([128, S], BF16, tag="kt")

        def cast_in(dst, src, lo, hi):
            nc.gpsimd.dma_start(
                out=dst[:, lo * D: hi * D].rearrange("p (t d) -> p t d", d=D),
                in_=src[lo * 128: hi * 128, j, :].rearrange(
                    "(t p) d -> p t d", p=128
                ),
            )

        def tr(dst, srcn, lo, hi):
            # blocked-transpose DMA: out 3D AP [d, t, s] -> the xbar emits
            # per-128x128-block transposes in a single transfer
            nc.sync.dma_start(
                out=dst[:, lo * 128: hi * 128].rearrange(
                    "p (t d) -> p t d", d=128
                ),
                in_=srcn[:, lo * 128: hi * 128],
                transpose=True,
            )

        # Q/K first (scores need them), V afterwards. The first pairs are
        # split into chunks so the first matmuls start earlier.
        nchunk = 2 if j == 0 else 1
        step = NT // nchunk
        for c in range(nchunk):
            lo, hi = c * step, (c + 1) * step
            cast_in(kn, self.k_d, lo, hi)
            cast_in(qn, self.q_d, lo, hi)
            tr(kt, kn, lo, hi)
            tr(qt, qn, lo, hi)
        cast_in(vn, self.v_d, 0, NT)
        self.qt[j], self.kt[j], self.vn[j] = qt, kt, vn

    def stage_A(self, j):
        """scoresT matmuls + exp (with 1.06 folded) + diagonal zeroing."""
        nc = self.nc
        qt, kt = self.qt[j], self.kt[j]
        et = []
        for kk in range(NT):
            q0 = kk * 128
            wk = S - q0
            e_kk = self.etp.tile([128, wk], BF16, tag=f"e{kk}")
            et.append(e_kk)
            kt_kk = kt[:, bass.ts(kk, 128)]
            # absolute-512-aligned q-groups, two per [128,1024] PSUM tile
            groups = list(range(kk // 4, 4))
            for i0 in range(0, len(groups), 2):
                gpair = groups[i0:i0 + 2]
                ps = self.psS.tile([128, 1024], F32, tag="ps_scores")
                base = gpair[0] * 512
                for g in gpair:
                    qlo = max(q0, g * 512)
                    nc.tensor.matmul(
                        ps[:, qlo - base: g * 512 - base + 512],
                        lhsT=kt_kk,
                        rhs=qt[:, qlo: g * 512 + 512],
                        start=True, stop=True,
                    )
                qlo0 = max(q0, base)
                wtot = gpair[-1] * 512 + 512 - qlo0
                nc.scalar.activation(
                    e_kk[:, qlo0 - q0: qlo0 - q0 + wtot],
                    ps[:, qlo0 - base: qlo0 - base + wtot],
                    mybir.ActivationFunctionType.Exp,
                    scale=float(INV_SQRT_D),
                    bias=self.bias_ln[:],
                )
                if i0 == 0:
                    # zero the k>q half of the diagonal block as soon as the
                    # first exp chunk (which contains it) lands
                    nc.gpsimd.affine_select(
                        out=e_kk[:, 0:128],
                        in_=e_kk[:, 0:128],
                        compare_op=mybir.AluOpType.is_ge,
                        fill=0.0,
                        base=0,
                        pattern=[[1, 128]],
                        channel_multiplier=-1,
                    )
        self.et[j] = et

    def stage_B(self, j):
        """Z' row-sums (ones-matmuls), Z copy-out, zlo broadcast."""
        nc = self.nc
        et = self.et[j]
        # z_row = (0.03/1.06) * Z'  (scale folded into the PSUM->SBUF copy);
        # the host recovers 1/Z as 0.03/z_row. z_row lives in row 0 of the
        # zlo broadcast tile.
        zlo = self.zbp.tile([128, S], F32, tag="zlo")
        z_row = zlo[0:1, :]
        for g in range(4):
            glo, ghi = g * 512, (g + 1) * 512
            zp = self.psZ.tile([1, 512], F32, tag="zp")
            kmax = 4 * g + 3
            for kk in range(kmax + 1):
                qlo = max(glo, kk * 128)
                nc.tensor.matmul(
                    zp[:, qlo - glo: 512],
                    lhsT=self.ones_k[:],
                    rhs=et[kk][:, qlo - kk * 128: ghi - kk * 128],
                    start=(kk == 0), stop=(kk == kmax),
                )
            nc.vector.tensor_scalar_mul(z_row[:, glo:ghi], zp[:, :], 0.03 / ALPHA)
            # broadcast row 0 across all 128 partitions on GPSIMD, per group
            nc.gpsimd.partition_broadcast(
                zlo[:, glo:ghi], zlo[0:1, glo:ghi]
            )
        nc.sync.dma_start(
            out=self.out_z[j].rearrange("(o s) -> o s", o=1), in_=zlo[0:1, :]
        )
        self.zlo[j] = zlo

    def stage_C(self, j):
        """Fused clip on DVE, then PV accumulation and output store."""
        nc = self.nc
        et, zlo, vn = self.et[j], self.zlo[j], self.vn[j]
        o_sb = self.osbp.tile([128, S], F32, tag="osb")
        tail = j == NP - 1
        clipped = 0
        for g in range(4):
            glo, ghi = g * 512, (g + 1) * 512
            kmax = 4 * g + 3
            if tail:
                # last pair: clip exactly the [kk, group] slices PV(g) needs
                # so the final clip->PV chain pipelines at group granularity
                for kk in range(kmax + 1):
                    qlo = max(glo, kk * 128)
                    nc.vector._custom_dve(
                        CLIPQ,
                        out=et[kk][:, qlo - kk * 128: ghi - kk * 128],
                        in0=et[kk][:, qlo - kk * 128: ghi - kk * 128],
                        in1=zlo[:, qlo:ghi],
                        imm2=KHI,
                    )
            else:
                # clip only the k-strips this PV group newly needs, so PV(g)
                # starts as soon as its prefix of clips is done
                for kk in range(clipped, kmax + 1):
                    nc.vector._custom_dve(
                        CLIPQ,
                        out=et[kk][:],
                        in0=et[kk][:],
                        in1=zlo[:, kk * 128: S],
                        imm2=KHI,
                    )
                clipped = kmax + 1
            op = self.psO.tile([128, 512], F32, tag="op")
            for kk in range(kmax + 1):
                qlo = max(glo, kk * 128)
                nc.tensor.matmul(
                    op[:, qlo - glo: 512],
                    lhsT=vn[:, bass.ts(kk, 128)],
                    rhs=et[kk][:, qlo - kk * 128: ghi - kk * 128],
                    start=(kk == 0), stop=(kk == kmax),
                )
            nc.scalar.copy(o_sb[:, glo:ghi], op[:, :])
            nc.sync.dma_start(
                out=self.out_t[j][:, glo:ghi], in_=o_sb[:, glo:ghi]
            )


_NC_CACHE = None


def _get_program():
    global _NC_CACHE
    if _NC_CACHE is None:
        _NC_CACHE = build_core_program()
    return _NC_CACHE


def kernel(query_states, key_states, value_states, batch_size, q_length, kv_length):
    assert int(batch_size) == B and int(q_length) == S and int(kv_length) == S
    qf = np.asarray(query_states, dtype=np.float32).reshape(B, S, H, D)
    kf = np.asarray(key_states, dtype=np.float32).reshape(B, S, H, D)
    vf = np.asarray(value_states, dtype=np.float32).reshape(B, S, H, D)

    nc = _get_program()

    in_maps = []
    for c in range(N_CORES):
        b = c // (N_CORES // B)
        h0 = NP * (c % (N_CORES // B))
        in_maps.append(
            {
                "q": np.ascontiguousarray(qf[b, :, h0:h0 + NP, :]),
                "k": np.ascontiguousarray(kf[b, :, h0:h0 + NP, :]),
                "v": np.ascontiguousarray(vf[b, :, h0:h0 + NP, :]),
            }
        )

    res = run_bass_kernel_spmd(nc, in_maps, list(range(N_CORES)))

    out = np.empty((B, S, H, D), dtype=np.float32)
    for c in range(N_CORES):
        b = c // (N_CORES // B)
        h0 = NP * (c % (N_CORES // B))
        ot = np.asarray(res.results[c]["out_t"])  # [NP, D, S]
        oz = np.asarray(res.results[c]["out_z"])  # [NP, S] = (0.03/1.06)*Z'
        for jj in range(NP):
            out[b, :, h0 + jj, :] = (ot[jj] * (0.03 / oz[jj])[None, :]).T
    return out.reshape(B * S, H, D)


# revision 50
# speedup vs baseline: 1.0401x; 1.0401x over previous
"""Trainium2 Bass kernel for causal self-attention with clipped softmax.

Problem (hardcoded): B=2, S=2048, H=16, D=128, fp32 inputs.
    scores = (Q @ K^T) / sqrt(D), causal mask, p = softmax(scores)
    p = clip(1.06*p - 0.03, 0, 1)            # ZETA=1.03, GAMMA=-0.03
    out = p @ V

Sharding: 32 (batch, head) pairs -> 4 per core across 8 cores (tensor
parallel over heads + data parallel over batch). No cross-core comms.

Per-core device kernel (transposed-scores layout, all matmuls bf16):
  - inputs loaded natural [s, d] with fp32->bf16 cast during DMA (SWDGE)
  - Q, K transposed to [d, s] via single blocked xbar-transpose DMAs
    (3D out AP -> sixteen 128x128 block transposes per transfer)
  - scoresT[k, q] = K_tile-stationary @ QT-moving  (causal tiles only)
  - exp on ScalarE directly from PSUM; scale=1/sqrt(D) and a 1.06 factor
    (bias=ln 1.06) folded in, so E' = 1.06*E and Z' = sum_k E' = 1.06*Z
  - causal diagonal zeroing via GPSIMD affine_select
  - Z'[q] via ones-column matmuls accumulated in PSUM; scaled to
    zlo = (0.03/1.06)*Z' during the PSUM drain, then broadcast across
    all 128 partitions with GPSIMD partition_broadcast
  - clipped numerator in ONE custom fused DVE op:
        G = relu(min(E' - zlo, 33.333*zlo))  (= Z * clip(1.06p - 0.03, 0, 1))
  - outT[d, q] += V_tile-stationary @ G-moving (PSUM accumulation over k)
  - host unshard applies the final (0.03/zlo_row) scale + layout transpose
The per-pair stages are emitted software-pipelined (A=scores/exp,
B=rowsum/broadcast, C=clip/PV with per-group clip prefixes) so the
in-order engine queues never head-of-line block across pairs.
"""

import numpy as np

import concourse.bass as bass
import concourse.mybir as mybir
import concourse.tile as tile
from concourse import bacc, dve_ops
from concourse.bass_utils import run_bass_kernel_spmd
from concourse.dve_spec import Spec, Src0, Src1, C2, lower, minn, relu
from concourse.dve_spec import _has_src1 as has_src1
from concourse.dve_uop import DveOpSpec

B = 2
S = 2048
H = 16
D = 128
N_CORES = 8
NP = H * B // N_CORES  # (b,h) pairs per core = 4
NT = S // 128  # 128-col tiles along sequence = 16
INV_SQRT_D = 1.0 / np.sqrt(np.float64(D))
ZETA = 1.03
GAMMA = -0.03
ALPHA = ZETA - GAMMA  # 1.06
KHI = 1.0 / 0.03  # zhi = KHI * zlo

F32 = mybir.dt.float32
BF16 = mybir.dt.bfloat16


def _register_clip_op():
    """Custom fused DVE op: out = relu(min(in0 - in1, imm2*in1)).

    With in1 = zlo = (0.03/1.06)*Z' and imm2 = 1/0.03 this computes the
    clipped-softmax numerator G = min(max(E' - 0.03Z, 0), Z) in a single
    DVE pass (sub, mul-by-imm, min, relu: 4 ALU stages, 2 streams).
    """
    name = "CLIPQ_ANT"
    for op in dve_ops.OPS:
        if op.name == name:
            return op
    spec = Spec(
        body=relu(minn(Src0 - Src1, Src1 * C2)),
        reference=lambda in0, in1, s0, s1, imm2: np.maximum(
            np.minimum(in0 - in1, in1 * imm2), 0.0
        ).astype(np.float32),
    )
    row = dve_ops._CUSTOM_DVE_ROW_BASE + len(dve_ops.OPS)
    dve_ops._SUB_OPCODE_FOR_NAME[name] = row
    shas = {}
    for ver in ("v3", "v4"):
        try:
            lowered = DveOpSpec(
                name=name,
                opcode=row,
                uops=lower(spec, ver=ver),
                rd1_en=has_src1(spec),
            )
            shas[ver] = lowered.sha(ver)
        except Exception:  # noqa: BLE001 - v4 table gen may be unavailable
            pass
    op = dve_ops.DveOp(name, spec, subdim=False, uops_sha=shas)
    dve_ops.OPS.append(op)
    dve_ops.CUSTOM_DVE_SPECS[name] = spec
    return op


CLIPQ = _register_clip_op()


def build_core_program():
    """Build + compile the per-core SPMD program. Returns the Bacc module."""
    nc = bacc.Bacc(
        "TRN2", target_bir_lowering=False, debug=False, num_devices=N_CORES
    )

    q_d = nc.dram_tensor("q", [S, NP, D], F32, kind="ExternalInput").ap()
    k_d = nc.dram_tensor("k", [S, NP, D], F32, kind="ExternalInput").ap()
    v_d = nc.dram_tensor("v", [S, NP, D], F32, kind="ExternalInput").ap()
    out_t = nc.dram_tensor("out_t", [NP, D, S], F32, kind="ExternalOutput").ap()
    out_z = nc.dram_tensor("out_z", [NP, S], F32, kind="ExternalOutput").ap()

    with tile.TileContext(nc) as tc:
        Builder(tc, q_d, k_d, v_d, out_t, out_z).build()

    nc.compile()
    return nc


class Builder:
    def __init__(self, tc, q_d, k_d, v_d, out_t, out_z):
        self.tc = tc
        self.nc = tc.nc
        self.q_d, self.k_d, self.v_d = q_d, k_d, v_d
        self.out_t, self.out_z = out_t, out_z
        self.qt = [None] * NP
        self.kt = [None] * NP
        self.vn = [None] * NP
        self.et = [None] * NP  # per pair: list per kk
        self.zlo = [None] * NP

    def build(self):
        nc = self.nc
        with (
            self.tc.tile_pool(name="const", bufs=1) as constp,
            self.tc.tile_pool(name="nat", bufs=2) as natp,
            self.tc.tile_pool(name="vnp", bufs=3) as vnp,
            self.tc.tile_pool(name="tr", bufs=2) as trp,
            self.tc.tile_pool(name="et", bufs=3) as etp,
            self.tc.tile_pool(name="zb", bufs=2) as zbp,
            self.tc.tile_pool(name="osb", bufs=2) as osbp,
            self.tc.tile_pool(name="psS", bufs=2, space="PSUM") as psS,
            self.tc.tile_pool(name="psZ", bufs=2, space="PSUM") as psZ,
            self.tc.tile_pool(name="psO", bufs=2, space="PSUM") as psO,
        ):
            self.natp, self.vnp, self.trp, self.etp = natp, vnp, trp, etp
            self.zbp, self.osbp = zbp, osbp
            self.psS, self.psZ, self.psO = psS, psZ, psO

            self.ones_k = constp.tile([128, 1], BF16)
            nc.vector.memset(self.ones_k[:], 1.0)
            self.bias_ln = constp.tile([128, 1], F32)
            nc.vector.memset(self.bias_ln[:], float(np.log(ALPHA)))

            # software pipeline over pairs: A=scores/exp, B=Z/bcast, C=clip/PV
            self.stage_in(0)
            self.stage_in(1)
            self.stage_A(0)
            self.stage_in(2)
            self.stage_A(1)
            self.stage_B(0)
            self.stage_B(1)
            self.stage_in(3)
            self.stage_A(2)
            self.stage_C(0)
            self.stage_B(2)
            self.stage_A(3)
            self.stage_C(1)
            self.stage_B(3)
            self.stage_C(2)
            self.stage_C(3)

    def stage_in(self, j):
        nc = self.nc
        qn = self.natp.tile([128, S], BF16, tag="qn")
        kn = self.natp.tile([128, S], BF16, tag="kn")
        vn = self.vnp.tile([128, S], BF16, tag="vn")
        qt = self.trp.tile([128, S], BF16, tag="qt")
        kt = self.trp.tile([128, S], BF16, tag="kt")

        def cast_in(dst, src, lo, hi):
            nc.gpsimd.dma_start(
                out=dst[:, lo * D: hi * D].rearrange("p (t d) -> p t d", d=D),
                in_=src[lo * 128: hi * 128, j, :].rearrange(
                    "(t p) d -> p t d", p=128
                ),
            )

        def tr(dst, srcn, lo, hi):
            # blocked-transpose DMA: out 3D AP [d, t, s] -> the xbar emits
            # per-128x128-block transposes in a single transfer
            nc.sync.dma_start(
                out=dst[:, lo * 128: hi * 128].rearrange(
                    "p (t d) -> p t d", d=128
                ),
                in_=srcn[:, lo * 128: hi * 128],
                transpose=True,
            )

        if j < 2:
            # Ramp pairs: HWDGE fp32 loads (no SWDGE issue latency) staged
            # through the not-yet-used output-buffer slots, bf16 convert on
            # the (idle) DVE, then transpose. Pair 0 in halves.
            nchunk = 2
            step = NT // nchunk
            for c in range(nchunk):
                lo, hi = c * step, (c + 1) * step
                for dst, srcd in ((kn, self.k_d), (qn, self.q_d)):
                    stg = self.osbp.tile(
                        [128, (hi - lo) * D], F32, tag="osb",
                        name=f"stg{j}_{c}_{dst.name[:1]}",
                    )
                    nc.sync.dma_start(
                        out=stg[:].rearrange("p (t d) -> p t d", d=D),
                        in_=srcd[lo * 128: hi * 128, j, :].rearrange(
                            "(t p) d -> p t d", p=128
                        ),
                    )
                    nc.vector.tensor_copy(dst[:, lo * D: hi * D], stg[:])
                tr(kt, kn, lo, hi)
                tr(qt, qn, lo, hi)
        else:
            cast_in(kn, self.k_d, 0, NT)
            cast_in(qn, self.q_d, 0, NT)
            tr(kt, kn, 0, NT)
            tr(qt, qn, 0, NT)
        # V is not needed until PV: cast it in stage_B instead
        self.vn[j] = vn
        self._vsrc = getattr(self, "_vsrc", {})
        self._vsrc[j] = (vn, cast_in)
        self.qt[j], self.kt[j] = qt, kt

    def stage_A(self, j):
        """scoresT matmuls + exp (with 1.06 folded) + diagonal zeroing."""
        nc = self.nc
        qt, kt = self.qt[j], self.kt[j]
        et = []
        for kk in range(NT):
            q0 = kk * 128
            wk = S - q0
            e_kk = self.etp.tile([128, wk], BF16, tag=f"e{kk}")
            et.append(e_kk)
            kt_kk = kt[:, bass.ts(kk, 128)]
            # absolute-512-aligned q-groups, two per [128,1024] PSUM tile
            groups = list(range(kk // 4, 4))
            for i0 in range(0, len(groups), 2):
                gpair = groups[i0:i0 + 2]
                ps = self.psS.tile([128, 1024], F32, tag="ps_scores")
                base = gpair[0] * 512
                for g in gpair:
                    qlo = max(q0, g * 512)
                    nc.tensor.matmul(
                        ps[:, qlo - base: g * 512 - base + 512],
                        lhsT=kt_kk,
                        rhs=qt[:, qlo: g * 512 + 512],
                        start=True, stop=True,
                    )
                qlo0 = max(q0, base)
                wtot = gpair[-1] * 512 + 512 - qlo0
                nc.scalar.activation(
                    e_kk[:, qlo0 - q0: qlo0 - q0 + wtot],
                    ps[:, qlo0 - base: qlo0 - base + wtot],
                    mybir.ActivationFunctionType.Exp,
                    scale=float(INV_SQRT_D),
                    bias=self.bias_ln[:],
                )
                if i0 == 0:
                    # zero the k>q half of the diagonal block as soon as the
                    # first exp chunk (which contains it) lands
                    nc.gpsimd.affine_select(
                        out=e_kk[:, 0:128],
                        in_=e_kk[:, 0:128],
                        compare_op=mybir.AluOpType.is_ge,
                        fill=0.0,
                        base=0,
                        pattern=[[1, 128]],
                        channel_multiplier=-1,
                    )
        self.et[j] = et

    def stage_B(self, j):
        """Z' row-sums (ones-matmuls), Z copy-out, zlo broadcast."""
        nc = self.nc
        et = self.et[j]
        vn, cast_v = self._vsrc[j]
        cast_v(vn, self.v_d, 0, NT)
        # z_row = (0.03/1.06) * Z'  (scale folded into the PSUM->SBUF copy);
        # the host recovers 1/Z as 0.03/z_row. z_row lives in row 0 of the
        # zlo broadcast tile.
        zlo = self.zbp.tile([128, S], F32, tag="zlo")
        z_row = zlo[0:1, :]
        for g in range(4):
            glo, ghi = g * 512, (g + 1) * 512
            zp = self.psZ.tile([1, 512], F32, tag="zp")
            kmax = 4 * g + 3
            for kk in range(kmax + 1):
                qlo = max(glo, kk * 128)
                nc.tensor.matmul(
                    zp[:, qlo - glo: 512],
                    lhsT=self.ones_k[:],
                    rhs=et[kk][:, qlo - kk * 128: ghi - kk * 128],
                    start=(kk == 0), stop=(kk == kmax),
                )
            nc.vector.tensor_scalar_mul(z_row[:, glo:ghi], zp[:, :], 0.03 / ALPHA)
            # broadcast row 0 across all 128 partitions on GPSIMD, per group
            nc.gpsimd.partition_broadcast(
                zlo[:, glo:ghi], zlo[0:1, glo:ghi]
            )
        nc.sync.dma_start(
            out=self.out_z[j].rearrange("(o s) -> o s", o=1), in_=zlo[0:1, :]
        )
        self.zlo[j] = zlo

    def stage_C(self, j):
        """Fused clip on DVE, then PV accumulation and output store."""
        nc = self.nc
        et, zlo, vn = self.et[j], self.zlo[j], self.vn[j]
        o_sb = self.osbp.tile([128, S], F32, tag="osb")
        tail = j == NP - 1
        clipped = 0
        for g in range(4):
            glo, ghi = g * 512, (g + 1) * 512
            kmax = 4 * g + 3
            if tail:
                # last pair: clip exactly the [kk, group] slices PV(g) needs
                # so the final clip->PV chain pipelines at group granularity
                for kk in range(kmax + 1):
                    qlo = max(glo, kk * 128)
                    nc.vector._custom_dve(
                        CLIPQ,
                        out=et[kk][:, qlo - kk * 128: ghi - kk * 128],
                        in0=et[kk][:, qlo - kk * 128: ghi - kk * 128],
                        in1=zlo[:, qlo:ghi],
                        imm2=KHI,
                    )
            else:
                # clip only the k-strips this PV group newly needs, so PV(g)
                # starts as soon as its prefix of clips is done
                for kk in range(clipped, kmax + 1):
                    nc.vector._custom_dve(
                        CLIPQ,
                        out=et[kk][:],
                        in0=et[kk][:],
                        in1=zlo[:, kk * 128: S],
                        imm2=KHI,
                    )
                clipped = kmax + 1
            op = self.psO.tile([128, 512], F32, tag="op")
            for kk in range(kmax + 1):
                qlo = max(glo, kk * 128)
                nc.tensor.matmul(
                    op[:, qlo - glo: 512],
                    lhsT=vn[:, bass.ts(kk, 128)],
                    rhs=et[kk][:, qlo - kk * 128: ghi - kk * 128],
                    start=(kk == 0), stop=(kk == kmax),
                )
            nc.scalar.copy(o_sb[:, glo:ghi], op[:, :])
            nc.sync.dma_start(
                out=self.out_t[j][:, glo:ghi], in_=o_sb[:, glo:ghi]
            )


_NC_CACHE = None


def _get_program():
    global _NC_CACHE
    if _NC_CACHE is None:
        _NC_CACHE = build_core_program()
    return _NC_CACHE


def kernel(query_states, key_states, value_states, batch_size, q_length, kv_length):
    assert int(batch_size) == B and int(q_length) == S and int(kv_length) == S
    qf = np.asarray(query_states, dtype=np.float32).reshape(B, S, H, D)
    kf = np.asarray(key_states, dtype=np.float32).reshape(B, S, H, D)
    vf = np.asarray(value_states, dtype=np.float32).reshape(B, S, H, D)

    nc = _get_program()

    in_maps = []
    for c in range(N_CORES):
        b = c // (N_CORES // B)
        h0 = NP * (c % (N_CORES // B))
        in_maps.append(
            {
                "q": np.ascontiguousarray(qf[b, :, h0:h0 + NP, :]),
                "k": np.ascontiguousarray(kf[b, :, h0:h0 + NP, :]),
                "v": np.ascontiguousarray(vf[b, :, h0:h0 + NP, :]),
            }
        )

    res = run_bass_kernel_spmd(nc, in_maps, list(range(N_CORES)))

    out = np.empty((B, S, H, D), dtype=np.float32)
    for c in range(N_CORES):
        b = c // (N_CORES // B)
        h0 = NP * (c % (N_CORES // B))
        ot = np.asarray(res.results[c]["out_t"])  # [NP, D, S]
        oz = np.asarray(res.results[c]["out_z"])  # [NP, S] = (0.03/1.06)*Z'
        for jj in range(NP):
            out[b, :, h0 + jj, :] = (ot[jj] * (0.03 / oz[jj])[None, :]).T
    return out.reshape(B * S, H, D)


# revision 53
# speedup vs baseline: 1.0487x; 1.0083x over previous
"""Trainium2 Bass kernel for causal self-attention with clipped softmax.

Problem (hardcoded): B=2, S=2048, H=16, D=128, fp32 inputs.
    scores = (Q @ K^T) / sqrt(D), causal mask, p = softmax(scores)
    p = clip(1.06*p - 0.03, 0, 1)            # ZETA=1.03, GAMMA=-0.03
    out = p @ V

Sharding: 32 (batch, head) pairs -> 4 per core across 8 cores (tensor
parallel over heads + data parallel over batch). No cross-core comms.

Per-core device kernel (transposed-scores layout, all matmuls bf16):
  - inputs loaded natural [s, d] with fp32->bf16 cast during DMA (SWDGE)
  - Q, K transposed to [d, s] via single blocked xbar-transpose DMAs
    (3D out AP -> sixteen 128x128 block transposes per transfer)
  - scoresT[k, q] = K_tile-stationary @ QT-moving  (causal tiles only)
  - exp on ScalarE directly from PSUM; scale=1/sqrt(D) and a 1.06 factor
    (bias=ln 1.06) folded in, so E' = 1.06*E and Z' = sum_k E' = 1.06*Z
  - causal diagonal zeroing via GPSIMD affine_select
  - Z'[q] via ones-column matmuls accumulated in PSUM; scaled to
    zlo = (0.03/1.06)*Z' during the PSUM drain, then broadcast across
    all 128 partitions with GPSIMD partition_broadcast
  - clipped numerator in ONE custom fused DVE op:
        G = relu(min(E' - zlo, 33.333*zlo))  (= Z * clip(1.06p - 0.03, 0, 1))
  - outT[d, q] += V_tile-stationary @ G-moving (PSUM accumulation over k)
  - host unshard applies the final (0.03/zlo_row) scale + layout transpose
The per-pair stages are emitted software-pipelined (A=scores/exp,
B=rowsum/broadcast, C=clip/PV with per-group clip prefixes) so the
in-order engine queues never head-of-line block across pairs.
"""

import numpy as np

import concourse.bass as bass
import concourse.mybir as mybir
import concourse.tile as tile
from concourse import bacc, dve_ops
from concourse.bass_utils import run_bass_kernel_spmd
from concourse.dve_spec import Spec, Src0, Src1, C2, lower, minn, relu
from concourse.dve_spec import _has_src1 as has_src1
from concourse.dve_uop import DveOpSpec

B = 2
S = 2048
H = 16
D = 128
N_CORES = 8
NP = H * B // N_CORES  # (b,h) pairs per core = 4
NT = S // 128  # 128-col tiles along sequence = 16
INV_SQRT_D = 1.0 / np.sqrt(np.float64(D))
ZETA = 1.03
GAMMA = -0.03
ALPHA = ZETA - GAMMA  # 1.06
KHI = 1.0 / 0.03  # zhi = KHI * zlo

F32 = mybir.dt.float32
BF16 = mybir.dt.bfloat16


def _register_clip_op():
    """Custom fused DVE op: out = relu(min(in0 - in1, imm2*in1)).

    With in1 = zlo = (0.03/1.06)*Z' and imm2 = 1/0.03 this computes the
    clipped-softmax numerator G = min(max(E' - 0.03Z, 0), Z) in a single
    DVE pass (sub, mul-by-imm, min, relu: 4 ALU stages, 2 streams).
    """
    name = "CLIPQ_ANT"
    for op in dve_ops.OPS:
        if op.name == name:
            return op
    spec = Spec(
        body=relu(minn(Src0 - Src1, Src1 * C2)),
        reference=lambda in0, in1, s0, s1, imm2: np.maximum(
            np.minimum(in0 - in1, in1 * imm2), 0.0
        ).astype(np.float32),
    )
    row = dve_ops._CUSTOM_DVE_ROW_BASE + len(dve_ops.OPS)
    dve_ops._SUB_OPCODE_FOR_NAME[name] = row
    shas = {}
    for ver in ("v3", "v4"):
        try:
            lowered = DveOpSpec(
                name=name,
                opcode=row,
                uops=lower(spec, ver=ver),
                rd1_en=has_src1(spec),
            )
            shas[ver] = lowered.sha(ver)
        except Exception:  # noqa: BLE001 - v4 table gen may be unavailable
            pass
    op = dve_ops.DveOp(name, spec, subdim=False, uops_sha=shas)
    dve_ops.OPS.append(op)
    dve_ops.CUSTOM_DVE_SPECS[name] = spec
    return op


CLIPQ = _register_clip_op()


def build_core_program():
    """Build + compile the per-core SPMD program. Returns the Bacc module."""
    nc = bacc.Bacc(
        "TRN2", target_bir_lowering=False, debug=False, num_devices=N_CORES
    )

    q_d = nc.dram_tensor("q", [S, NP, D], F32, kind="ExternalInput").ap()
    k_d = nc.dram_tensor("k", [S, NP, D], F32, kind="ExternalInput").ap()
    v_d = nc.dram_tensor("v", [S, NP, D], F32, kind="ExternalInput").ap()
    out_t = nc.dram_tensor("out_t", [NP, D, S], F32, kind="ExternalOutput").ap()
    out_z = nc.dram_tensor("out_z", [NP, S], F32, kind="ExternalOutput").ap()

    with tile.TileContext(nc) as tc:
        Builder(tc, q_d, k_d, v_d, out_t, out_z).build()

    nc.compile()
    return nc


class Builder:
    def __init__(self, tc, q_d, k_d, v_d, out_t, out_z):
        self.tc = tc
        self.nc = tc.nc
        self.q_d, self.k_d, self.v_d = q_d, k_d, v_d
        self.out_t, self.out_z = out_t, out_z
        self.qt = [None] * NP
        self.kt = [None] * NP
        self.vn = [None] * NP
        self.et = [None] * NP  # per pair: list per kk
        self.zlo = [None] * NP

    def build(self):
        nc = self.nc
        with (
            self.tc.tile_pool(name="const", bufs=1) as constp,
            self.tc.tile_pool(name="nat", bufs=2) as natp,
            self.tc.tile_pool(name="vnp", bufs=3) as vnp,
            self.tc.tile_pool(name="tr", bufs=2) as trp,
            self.tc.tile_pool(name="et", bufs=3) as etp,
            self.tc.tile_pool(name="zb", bufs=2) as zbp,
            self.tc.tile_pool(name="osb", bufs=2) as osbp,
            self.tc.tile_pool(name="psS", bufs=2, space="PSUM") as psS,
            self.tc.tile_pool(name="psZ", bufs=2, space="PSUM") as psZ,
            self.tc.tile_pool(name="psO", bufs=2, space="PSUM") as psO,
        ):
            self.natp, self.vnp, self.trp, self.etp = natp, vnp, trp, etp
            self.zbp, self.osbp = zbp, osbp
            self.psS, self.psZ, self.psO = psS, psZ, psO

            self.ones_k = constp.tile([128, 128], BF16)
            nc.vector.memset(self.ones_k[:], 1.0)
            self.bias_ln = constp.tile([128, 1], F32)
            nc.vector.memset(self.bias_ln[:], float(np.log(ALPHA)))

            # software pipeline over pairs: A=scores/exp, B=Z/bcast, C=clip/PV
            self.stage_in(0)
            self.stage_in(1)
            self.stage_A(0)
            self.stage_in(2)
            self.stage_A(1)
            self.stage_B(0)
            self.stage_B(1)
            self.stage_in(3)
            self.stage_A(2)
            self.stage_C(0)
            self.stage_B(2)
            self.stage_A(3)
            self.stage_C(1)
            self.stage_B(3)
            self.stage_C(2)
            self.stage_C(3)

    def stage_in(self, j):
        nc = self.nc
        qn = self.natp.tile([128, S], BF16, tag="qn")
        kn = self.natp.tile([128, S], BF16, tag="kn")
        vn = self.vnp.tile([128, S], BF16, tag="vn")
        qt = self.trp.tile([128, S], BF16, tag="qt")
        kt = self.trp.tile([128, S], BF16, tag="kt")

        def cast_in(dst, src, lo, hi):
            nc.gpsimd.dma_start(
                out=dst[:, lo * D: hi * D].rearrange("p (t d) -> p t d", d=D),
                in_=src[lo * 128: hi * 128, j, :].rearrange(
                    "(t p) d -> p t d", p=128
                ),
            )

        def tr(dst, srcn, lo, hi):
            # blocked-transpose DMA: out 3D AP [d, t, s] -> the xbar emits
            # per-128x128-block transposes in a single transfer
            nc.sync.dma_start(
                out=dst[:, lo * 128: hi * 128].rearrange(
                    "p (t d) -> p t d", d=128
                ),
                in_=srcn[:, lo * 128: hi * 128],
                transpose=True,
            )

        if j < 2:
            # Ramp pairs: HWDGE fp32 loads (no SWDGE issue latency) staged
            # through the not-yet-used output-buffer slots, bf16 convert on
            # the (idle) DVE, then transpose. Pair 0 in halves.
            nchunk = 2
            step = NT // nchunk
            for c in range(nchunk):
                lo, hi = c * step, (c + 1) * step
                for dst, srcd in ((kn, self.k_d), (qn, self.q_d)):
                    stg = self.osbp.tile(
                        [128, (hi - lo) * D], F32, tag="osb",
                        name=f"stg{j}_{c}_{dst.name[:1]}",
                    )
                    nc.sync.dma_start(
                        out=stg[:].rearrange("p (t d) -> p t d", d=D),
                        in_=srcd[lo * 128: hi * 128, j, :].rearrange(
                            "(t p) d -> p t d", p=128
                        ),
                    )
                    nc.vector.tensor_copy(dst[:, lo * D: hi * D], stg[:])
                tr(kt, kn, lo, hi)
                tr(qt, qn, lo, hi)
        else:
            cast_in(kn, self.k_d, 0, NT)
            cast_in(qn, self.q_d, 0, NT)
            tr(kt, kn, 0, NT)
            tr(qt, qn, 0, NT)
        # V is not needed until PV: cast it in stage_B instead
        self.vn[j] = vn
        self._vsrc = getattr(self, "_vsrc", {})
        self._vsrc[j] = (vn, cast_in)
        self.qt[j], self.kt[j] = qt, kt

    def stage_A(self, j):
        """scoresT matmuls + exp (with 1.06 folded) + diagonal zeroing."""
        nc = self.nc
        qt, kt = self.qt[j], self.kt[j]
        et = []
        for kk in range(NT):
            q0 = kk * 128
            wk = S - q0
            e_kk = self.etp.tile([128, wk], BF16, tag=f"e{kk}")
            et.append(e_kk)
            kt_kk = kt[:, bass.ts(kk, 128)]
            # absolute-512-aligned q-groups, two per [128,1024] PSUM tile
            groups = list(range(kk // 4, 4))
            for i0 in range(0, len(groups), 2):
                gpair = groups[i0:i0 + 2]
                ps = self.psS.tile([128, 1024], F32, tag="ps_scores")
                base = gpair[0] * 512
                for g in gpair:
                    qlo = max(q0, g * 512)
                    nc.tensor.matmul(
                        ps[:, qlo - base: g * 512 - base + 512],
                        lhsT=kt_kk,
                        rhs=qt[:, qlo: g * 512 + 512],
                        start=True, stop=True,
                    )
                qlo0 = max(q0, base)
                wtot = gpair[-1] * 512 + 512 - qlo0
                nc.scalar.activation(
                    e_kk[:, qlo0 - q0: qlo0 - q0 + wtot],
                    ps[:, qlo0 - base: qlo0 - base + wtot],
                    mybir.ActivationFunctionType.Exp,
                    scale=float(INV_SQRT_D),
                    bias=self.bias_ln[:],
                )
                if i0 == 0:
                    # zero the k>q half of the diagonal block as soon as the
                    # first exp chunk (which contains it) lands
                    nc.gpsimd.affine_select(
                        out=e_kk[:, 0:128],
                        in_=e_kk[:, 0:128],
                        compare_op=mybir.AluOpType.is_ge,
                        fill=0.0,
                        base=0,
                        pattern=[[1, 128]],
                        channel_multiplier=-1,
                    )
        self.et[j] = et

    def stage_B(self, j):
        """Z' row-sums (ones-matmuls), Z copy-out, zlo broadcast."""
        nc = self.nc
        et = self.et[j]
        vn, cast_v = self._vsrc[j]
        cast_v(vn, self.v_d, 0, NT)
        # z_row = (0.03/1.06) * Z'  (scale folded into the PSUM->SBUF copy);
        # the host recovers 1/Z as 0.03/z_row. z_row lives in row 0 of the
        # zlo broadcast tile.
        zlo = self.zbp.tile([128, S], F32, tag="zlo")
        for g in range(4):
            glo, ghi = g * 512, (g + 1) * 512
            # all-ones [128,128] stationary -> the rowsum matmul itself emits
            # Z replicated on every partition (the broadcast tile), free
            zp = self.psZ.tile([128, 512], F32, tag="zp")
            kmax = 4 * g + 3
            for kk in range(kmax + 1):
                qlo = max(glo, kk * 128)
                nc.tensor.matmul(
                    zp[:, qlo - glo: 512],
                    lhsT=self.ones_k[:],
                    rhs=et[kk][:, qlo - kk * 128: ghi - kk * 128],
                    start=(kk == 0), stop=(kk == kmax),
                )
            nc.vector.tensor_scalar_mul(zlo[:, glo:ghi], zp[:, :], 0.03 / ALPHA)
        nc.sync.dma_start(
            out=self.out_z[j].rearrange("(o s) -> o s", o=1), in_=zlo[0:1, :]
        )
        self.zlo[j] = zlo

    def stage_C(self, j):
        """Fused clip on DVE, then PV accumulation and output store."""
        nc = self.nc
        et, zlo, vn = self.et[j], self.zlo[j], self.vn[j]
        o_sb = self.osbp.tile([128, S], F32, tag="osb")
        tail = j == NP - 1
        clipped = 0
        for g in range(4):
            glo, ghi = g * 512, (g + 1) * 512
            kmax = 4 * g + 3
            if tail:
                # last pair: clip exactly the [kk, group] slices PV(g) needs
                # so the final clip->PV chain pipelines at group granularity
                for kk in range(kmax + 1):
                    qlo = max(glo, kk * 128)
                    nc.vector._custom_dve(
                        CLIPQ,
                        out=et[kk][:, qlo - kk * 128: ghi - kk * 128],
                        in0=et[kk][:, qlo - kk * 128: ghi - kk * 128],
                        in1=zlo[:, qlo:ghi],
                        imm2=KHI,
                    )
            else:
                # clip only the k-strips this PV group newly needs, so PV(g)
                # starts as soon as its prefix of clips is done
                for kk in range(clipped, kmax + 1):
                    nc.vector._custom_dve(
                        CLIPQ,
                        out=et[kk][:],
                        in0=et[kk][:],
                        in1=zlo[:, kk * 128: S],
                        imm2=KHI,
                    )
                clipped = kmax + 1
            op = self.psO.tile([128, 512], F32, tag="op")
            for kk in range(kmax + 1):
                qlo = max(glo, kk * 128)
                nc.tensor.matmul(
                    op[:, qlo - glo: 512],
                    lhsT=vn[:, bass.ts(kk, 128)],
                    rhs=et[kk][:, qlo - kk * 128: ghi - kk * 128],
                    start=(kk == 0), stop=(kk == kmax),
                )
            nc.scalar.copy(o_sb[:, glo:ghi], op[:, :])
            nc.sync.dma_start(
                out=self.out_t[j][:, glo:ghi], in_=o_sb[:, glo:ghi]
            )


_NC_CACHE = None


def _get_program():
    global _NC_CACHE
    if _NC_CACHE is None:
        _NC_CACHE = build_core_program()
    return _NC_CACHE


def kernel(query_states, key_states, value_states, batch_size, q_length, kv_length):
    assert int(batch_size) == B and int(q_length) == S and int(kv_length) == S
    qf = np.asarray(query_states, dtype=np.float32).reshape(B, S, H, D)
    kf = np.asarray(key_states, dtype=np.float32).reshape(B, S, H, D)
    vf = np.asarray(value_states, dtype=np.float32).reshape(B, S, H, D)

    nc = _get_program()

    in_maps = []
    for c in range(N_CORES):
        b = c // (N_CORES // B)
        h0 = NP * (c % (N_CORES // B))
        in_maps.append(
            {
                "q": np.ascontiguousarray(qf[b, :, h0:h0 + NP, :]),
                "k": np.ascontiguousarray(kf[b, :, h0:h0 + NP, :]),
                "v": np.ascontiguousarray(vf[b, :, h0:h0 + NP, :]),
            }
        )

    res = run_bass_kernel_spmd(nc, in_maps, list(range(N_CORES)))

    out = np.empty((B, S, H, D), dtype=np.float32)
    for c in range(N_CORES):
        b = c // (N_CORES // B)
        h0 = NP * (c % (N_CORES // B))
        ot = np.asarray(res.results[c]["out_t"])  # [NP, D, S]
        oz = np.asarray(res.results[c]["out_z"])  # [NP, S] = (0.03/1.06)*Z'
        for jj in range(NP):
            out[b, :, h0 + jj, :] = (ot[jj] * (0.03 / oz[jj])[None, :]).T
    return out.reshape(B * S, H, D)
